# revision 39
# baseline (speedup 1.0000x reference)
"""3-layer GAT (DGL GATConv, 1 head) on Trainium2, sharded over 8 NeuronCores.

Strategy (graph/data parallel by destination node):
  * Nodes split into 8 blocks of 6250; each core owns edges whose dst falls in
    its block.  Within a core, dsts are bin-packed into 49 supertiles of <=128
    dsts each, balancing per-supertile edge counts (NJ = max blocks-of-128
    edges drops to ~the mean).  The node order within a core is therefore a
    permutation; all index remapping happens on host, the device only sees
    contiguous permuted rows, and the host unpermutes the final output.
  * Per layer each core computes a bf16 node-table shard with 512B rows
    [ft (f_out), 1.0, el, pad] (ft = h @ W, el = h @ (W @ al)); er = h@(W@ar)
    is kept in a local fp32 column.  Shards are AllGathered so every core
    holds the full 50000-row table in DRAM.
  * Edge stage per supertile: one big SWDGE dma_gather fetches all edge-source
    rows (512B each) in <=1024-index chunks — the int16 index limit splits the
    table into a low half (rows < 32768) and a high half (offset base).
    This replaces the per-128-row indirect DMA hot loop of the previous
    version (~1us fixed gpsimd cost per instruction).
  * ex = exp(leaky_relu(el_src + er_dst)); er_dst is broadcast to all
    partitions via a rank-1 matmul.  A one-hot mask (iota == shift) times ex
    forms the scatter matrix S; matmuls accumulate
    psum[128 dsts, f_out+1] += S_j^T @ X_j; the table's constant 1.0 column
    makes psum's last column the softmax denominator.  Softmax max-subtraction
    is skipped: logits are O(10), exp cannot overflow, result identical.
  * Epilogue: out = relu(psum * 1/esum) written to the core's permuted
    next-layer h block; layer 3 writes the external output shard (fp32).

Self-contained: hardcodes the problem shapes; host-side work is numpy only.
"""

import numpy as np
import ml_dtypes

import concourse.bass as bass
import concourse.mybir as mybir
import concourse.tile as tile
from concourse import bacc
from concourse.bass_utils import run_bass_kernel_spmd
from concourse.masks import make_identity

FP = mybir.dt.float32
BF = mybir.dt.bfloat16
I16 = mybir.dt.int16

BF_NP = ml_dtypes.bfloat16

N_NODES = 50000
N_CORES = 8
IN_F = 128
NEG_SLOPE = 0.2
SUP = 128           # dsts per supertile (= psum rows)
SPLIT = 32768       # int16 gather index limit: table low/high split row
CHUNK = 1024        # SWDGE descriptor-ring capacity (indices per dma_gather)
G = 2               # supertiles batched per edge-stage iteration


class _Cfg:
    def __init__(self, n_nodes, n_cores, feats, jl, jh):
        self.N = n_nodes
        self.C = n_cores
        self.NB = n_nodes // n_cores
        assert self.NB * n_cores == n_nodes
        self.feats = feats  # list of (F_in, F_out) per layer
        self.NSUP = -(-self.NB // SUP)
        self.JL = jl        # low-half blocks of 128 slots per supertile
        self.JH = jh        # high-half blocks
        self.NJ = jl + jh

    def table_width(self, li):
        # bf16 row [ft (f_out), one, el, pad...]: 512B rows for f_out=128,
        # 256B rows for f_out=16 (dma_gather needs elem_size % 256B == 0)
        f = self.feats[li][1]
        return 256 if f + 2 > 128 else 128

    @property
    def NITER(self):
        return -(-self.NSUP // G)

    def key(self):
        return (self.N, self.C, tuple(self.feats), self.JL, self.JH)


def _pack_edges(src, dst, cfg_probe):
    """Sort edges by dst block, bin-pack dsts into supertiles, split lo/hi.

    Returns (jl, jh, meta, perms): meta is int16
    [C, NITER, 128, G*NJ*8 + G*NJ] holding, per G-supertile iteration, the
    wrapped SWDGE gather indices for each supertile followed by the one-hot
    shift values as bf16 bits.  perms[c] maps device row -> original local
    dst id.
    """
    C, NB, NSUP = cfg_probe.C, cfg_probe.NB, cfg_probe.NSUP
    order = np.argsort(dst, kind="stable")
    src_s = src[order].astype(np.int64)
    dst_s = dst[order].astype(np.int64)
    core_lo = np.searchsorted(dst_s, np.arange(C) * NB)
    core_hi = np.searchsorted(dst_s, (np.arange(C) + 1) * NB)

    perms = np.zeros((C, NB), np.int64)
    percore = []
    jl = jh = 0
    for c in range(C):
        lo, hi = core_lo[c], core_hi[c]
        dloc = dst_s[lo:hi] - c * NB          # local dst of each edge
        sloc = src_s[lo:hi]
        deg = np.bincount(dloc, minlength=NB)
        # greedy balanced bin packing: heaviest dst first into lightest
        # feasible bin (<=128 dsts; bin NSUP-1 also capped to fit NB total)
        cap = np.full(NSUP, SUP, np.int64)
        cap[-1] = NB - (NSUP - 1) * SUP
        load = np.zeros(NSUP, np.int64)
        fill = np.zeros(NSUP, np.int64)
        bin_of = np.zeros(NB, np.int64)
        pos_of = np.zeros(NB, np.int64)
        for d in np.argsort(-deg, kind="stable"):
            b = np.argmin(np.where(fill < cap, load, np.iinfo(np.int64).max))
            bin_of[d] = b
            pos_of[d] = fill[b]
            fill[b] += 1
            load[b] += deg[d]
        # device row of original local dst d = bin_of*SUP + pos_of
        # (bins 0..NSUP-2 are full 128-dst bins; the last holds the rest)
        dev_row = bin_of * SUP + pos_of
        perm = np.zeros(NB, np.int64)
        perm[dev_row] = np.arange(NB)
        perms[c] = perm

        e_sup = bin_of[dloc]                  # supertile of each edge
        e_w = pos_of[dloc]                    # dst slot within supertile
        e_lo = sloc < SPLIT
        nlo = np.zeros(NSUP, np.int64)
        nhi = np.zeros(NSUP, np.int64)
        np.add.at(nlo, e_sup[e_lo], 1)
        np.add.at(nhi, e_sup[~e_lo], 1)
        jl = max(jl, -(-int(nlo.max()) // SUP))
        jh = max(jh, -(-int(nhi.max()) // SUP))
        percore.append((sloc, e_sup, e_w, e_lo))

    nj = jl + jh
    niter = -(-NSUP // G)
    shoff = G * nj * 8
    meta = np.zeros((C, niter, 128, G * nj * 9), np.int16)
    pad_bits = np.asarray(128.0, BF_NP).view(np.int16)
    meta[:, :, :, shoff:] = pad_bits
    plan = _chunk_plan(jl, jh)
    for c in range(C):
        sloc, e_sup, e_w, e_lo = percore[c]
        # slot index within supertile: low edges pack from 0, high edges
        # pack from JL*128
        key = e_sup * 4 + (~e_lo).astype(np.int64)  # group (sup, half)
        grp_order = np.argsort(key, kind="stable")
        key_sorted = key[grp_order]
        gstart = np.searchsorted(key_sorted, key)
        slot_in_grp = np.empty(len(key), np.int64)
        slot_in_grp[grp_order] = np.arange(len(key)) - gstart[grp_order]
        slot = np.where(e_lo, slot_in_grp, jl * 128 + slot_in_grp)
        flat_idx = np.zeros((NSUP, nj * 128), np.int64)
        val = np.where(e_lo, sloc, sloc - SPLIT)
        flat_idx[e_sup, slot] = val
        shift = np.full((NSUP, nj * 128), 128.0, np.float32)
        shift[e_sup, slot] = e_w[...]
        # wrapped SWDGE layout per gather chunk: within a chunk, index i'
        # sits at [16g + i'%16, chunk_col0 + i'//16] for g in 0..7
        for s in range(NSUP):
            it, gg = divmod(s, G)
            row16 = np.zeros((16, nj * 8), np.int16)
            for b0, nb, _lo, _q in plan:
                vals = flat_idx[s, b0 * 128:(b0 + nb) * 128]
                row16[:, b0 * 8:(b0 + nb) * 8] = (
                    vals.reshape(nb * 8, 16).T.astype(np.int16))
            meta[c, it, :, gg * nj * 8:(gg + 1) * nj * 8] = \
                np.tile(row16, (8, 1))
            # shift [k, j] = shift[s, j*128+k], as bf16 bits
            sh = np.asarray(shift[s].reshape(nj, 128).T,
                            BF_NP).view(np.int16)
            meta[c, it, :, shoff + gg * nj:shoff + (gg + 1) * nj] = sh
    return jl, jh, meta, perms


def _chunk_plan(jl, jh):
    """Static dma_gather chunking balanced over the 4 SWDGE queues.

    Returns list of (block0, nblocks, is_low, queue).  Descriptor generation
    runs in parallel across queues, so spread the NJ blocks evenly ([9,9,8,8]
    style), splitting instructions at the 1024-descriptor ring limit and at
    the low/high table boundary.
    """
    nj = jl + jh
    per_q = [(nj + 3 - q) // 4 for q in range(4)]  # blocks per queue
    plan = []
    b = 0
    for q in range(4):
        left = per_q[q]
        while left > 0:
            # blocks [b, b+n) must stay within one half and one ring window
            half_end = jl if b < jl else nj
            n = min(left, CHUNK // 128, half_end - b)
            plan.append((b, n, b < jl, q))
            b += n
            left -= n
    assert b == nj
    return plan


def _build(cfg, has_bias):
    """Build + compile the (core-independent) Bass program."""
    nc = bacc.Bacc(
        "TRN2",
        target_bir_lowering=False,
        debug=False,
        num_devices=cfg.C,
        num_swdge_queues=4,
    )
    NB, NSUP, NJ, JL, JH = cfg.NB, cfg.NSUP, cfg.NJ, cfg.JL, cfg.JH
    NL = len(cfg.feats)

    NITER = cfg.NITER
    SHOFF = G * NJ * 8
    feat_c = nc.dram_tensor("feat_c", [NB, IN_F], BF, kind="ExternalInput")
    iota_in = nc.dram_tensor("iota", [128, SUP], BF, kind="ExternalInput")
    meta_in = nc.dram_tensor("meta", [NITER, 128, G * NJ * 9], I16,
                             kind="ExternalInput")
    waug = [
        nc.dram_tensor(f"waug{li}", [cfg.feats[li][0], cfg.feats[li][1] + 2],
                       BF, kind="ExternalInput")
        for li in range(NL)
    ]
    bias_in = [
        nc.dram_tensor(f"bias{li}", [128, cfg.feats[li][1]], FP,
                       kind="ExternalInput")
        if has_bias[li] else None
        for li in range(NL)
    ]

    tbl_shard = [
        nc.dram_tensor(f"tbl_shard{li}", [NB, cfg.table_width(li)], BF)
        for li in range(NL)
    ]
    shared_kw = {"addr_space": "Shared"} if cfg.C > 4 else {}
    tbl_full = [
        nc.dram_tensor(f"tbl_full{li}", [cfg.N, cfg.table_width(li)], BF,
                       **shared_kw)
        for li in range(NL)
    ]
    er_own = [nc.dram_tensor(f"er_own{li}", [NB, 1], BF) for li in range(NL)]
    h_mid = [
        nc.dram_tensor(f"h_mid{li}", [NB, cfg.feats[li][1]], BF)
        for li in range(NL - 1)
    ]
    out_c = nc.dram_tensor("out_c", [NB, cfg.feats[-1][1]], FP,
                           kind="ExternalOutput")

    n_row_tiles = -(-NB // 128)
    replica = [list(range(cfg.C))]
    plan = _chunk_plan(JL, JH)

    with tile.TileContext(nc, num_cores=cfg.C) as tc:
        with (
            tc.tile_pool(name="const", bufs=1) as constp,
            tc.tile_pool(name="nodein", bufs=3) as nodein,
            tc.tile_pool(name="nodet", bufs=2) as nodet,
            tc.tile_pool(name="nodepsum", bufs=1, space="PSUM") as nodepsum,
            tc.tile_pool(name="stage", bufs=4) as stagep,
            tc.tile_pool(name="erp", bufs=1) as erp,
            tc.tile_pool(name="idx", bufs=4) as idxp,
            tc.tile_pool(name="xg", bufs=2) as xgp,
            tc.tile_pool(name="aex", bufs=1) as aexp,
            tc.tile_pool(name="sm", bufs=2) as smp,
            tc.tile_pool(name="ebp", bufs=2, space="PSUM") as ebp,
            tc.tile_pool(name="epsum", bufs=2, space="PSUM") as epsum,
            tc.tile_pool(name="eout", bufs=4) as eoutp,
        ):
            ident = constp.tile([128, 128], BF, tag="ident")
            make_identity(nc, ident[:])
            iota_sb = constp.tile([128, SUP], BF, tag="iota")
            nc.sync.dma_start(out=iota_sb[:], in_=iota_in[:])
            ones_row = constp.tile([1, SUP], BF, tag="ones")
            nc.vector.memset(ones_row[:], 1.0)
            ones_col = constp.tile([128, 1], BF, tag="ones_col")
            nc.vector.memset(ones_col[:], 1.0)

            for li in range(NL):
                f_in, f_out = cfg.feats[li]
                tw = cfg.table_width(li)

                wsb = constp.tile([f_in, f_out + 2], BF, tag=f"waug{li}")
                nc.sync.dma_start(out=wsb[:], in_=waug[li][:])
                if has_bias[li]:
                    bsb = constp.tile([128, f_out], FP, tag=f"bias{li}")
                    nc.sync.dma_start(out=bsb[:], in_=bias_in[li][:])

                # ---- node stage: own block rows -> table shard + er column
                # ps2 is a bf16 psum tile [ft, el, er]; the table row
                # [ft, one, el] is written straight from psum by DMA (no
                # Act copies): ft -> cols 0:f_out, el -> col f_out+1,
                # er -> er_own, and the constant-one column from a tile.
                hsrc = feat_c if li == 0 else h_mid[li - 1]
                for t in range(n_row_tiles):
                    r0 = t * 128
                    rows = min(128, NB - r0)
                    h_t = nodein.tile([128, f_in], BF, tag="h")
                    nc.sync.dma_start(out=h_t[:rows], in_=hsrc[r0:r0 + rows, :])
                    ps_t = nodepsum.tile([f_in, 128], BF, tag="pT")
                    nc.tensor.transpose(out=ps_t[:, :rows], in_=h_t[:rows],
                                        identity=ident[:rows, :rows])
                    hT = nodet.tile([f_in, 128], BF, tag="hT")
                    nc.scalar.copy(out=hT[:, :rows], in_=ps_t[:, :rows])
                    ps2 = nodepsum.tile([128, f_out + 2], FP, tag="p2")
                    nc.tensor.matmul(out=ps2[:rows], lhsT=hT[:, :rows],
                                     rhs=wsb[:], start=True, stop=True)
                    st = stagep.tile([128, f_out + 2], BF, tag="st")
                    nc.scalar.copy(out=st[:rows], in_=ps2[:rows])
                    nc.sync.dma_start(
                        out=tbl_shard[li][r0:r0 + rows, 0:f_out],
                        in_=st[:rows, 0:f_out])
                    nc.sync.dma_start(
                        out=tbl_shard[li][r0:r0 + rows, f_out + 1:f_out + 2],
                        in_=st[:rows, f_out:f_out + 1])
                    nc.sync.dma_start(
                        out=tbl_shard[li][r0:r0 + rows, f_out:f_out + 1],
                        in_=ones_col[:rows])
                    nc.sync.dma_start(out=er_own[li][r0:r0 + rows, :],
                                      in_=st[:rows, f_out + 1:f_out + 2])

                # ---- all-gather the node table ----
                if "no_collective" not in _ABL:
                    nc.gpsimd.collective_compute(
                        "AllGather",
                        mybir.AluOpType.bypass,
                        replica_groups=replica,
                        ins=[tbl_shard[li][:]],
                        outs=[tbl_full[li][:]],
                    )

                abl = set(_ABL.split("+"))
                # per-layer er preload: one [1, NITER*G*SUP] row, padded 0
                er_all = erp.tile([1, NITER * G * SUP], BF, tag="er_all")
                nc.vector.memset(er_all[:], 0.0)
                nc.sync.dma_start(out=er_all[:1, 0:NB],
                                  in_=er_own[li][:, 0][None, :])

                last = li == NL - 1
                el_c = f_out + 1
                psw = f_out + 1
                # ---- edge stage: G supertiles per iteration ----
                for it in range(NITER):
                    s0 = it * G
                    meta_t = idxp.tile([128, G * NJ * 9], I16, tag="meta")
                    nc.sync.dma_start(out=meta_t[:], in_=meta_in[it])
                    x_t = xgp.tile([128, G * NJ, tw], BF, tag="x")
                    if "no_gather" in abl:
                        nc.vector.memset(
                            x_t[:].rearrange("p j c -> p (j c)"), 0.0)
                    for gg in range(G):
                        if "no_gather" in abl:
                            break
                        for ci, (b0, nb, is_lo, qn) in enumerate(plan):
                            tbl_ap = (tbl_full[li][0:SPLIT, :] if is_lo
                                      else tbl_full[li][SPLIT:cfg.N, :])
                            nc.gpsimd.dma_gather(
                                out_ap=x_t[:, gg * NJ + b0:gg * NJ + b0 + nb,
                                           :],
                                in_ap=tbl_ap,
                                idxs_ap=meta_t[:, gg * NJ * 8 + b0 * 8:
                                               gg * NJ * 8 + (b0 + nb) * 8],
                                num_idxs=nb * 128,
                                num_idxs_reg=nb * 128,
                                elem_size=tw,
                                queue_num=(qn + 2 * gg) % 4,
                            )

                    # er for these supertiles' dsts -> all partitions
                    eb_ps = ebp.tile([128, G * SUP], FP, tag="eb")
                    nc.tensor.matmul(
                        out=eb_ps[:], lhsT=ones_row[:],
                        rhs=er_all[0:1, s0 * SUP:(s0 + G) * SUP],
                        start=True, stop=True)

                    # A = el_src + er_dst, per supertile (er varies by g)
                    a_t = aexp.tile([128, G * NJ * SUP], BF, tag="a")
                    if "no_ex" not in abl:
                        for gg in range(G):
                            a3 = a_t[:, gg * NJ * SUP:(gg + 1) * NJ * SUP] \
                                .rearrange("p (j w) -> p j w", w=SUP)
                            nc.vector.tensor_tensor(
                                out=a3,
                                in0=x_t[:, gg * NJ:(gg + 1) * NJ,
                                        el_c:el_c + 1].to_broadcast(
                                    [128, NJ, SUP]),
                                in1=eb_ps[:, None,
                                          gg * SUP:(gg + 1) * SUP]
                                .to_broadcast([128, NJ, SUP]),
                                op=mybir.AluOpType.add,
                            )
                    # M = (iota == shift)
                    m_t = smp.tile([128, G * NJ * SUP], BF, tag="m")
                    m3 = m_t[:].rearrange("p (j w) -> p j w", w=SUP)
                    nc.vector.tensor_tensor(
                        out=m3,
                        in0=iota_sb[:, None, :].to_broadcast(
                            [128, G * NJ, SUP]),
                        in1=meta_t[:, SHOFF:].bitcast(BF)[:, :, None]
                        .to_broadcast([128, G * NJ, SUP]),
                        op=mybir.AluOpType.is_equal,
                    )
                    if "no_ex" not in abl:
                        # EX = exp(leaky_relu(A)) on the scalar engine
                        ex_t = aexp.tile([128, G * NJ * SUP], BF, tag="exv")
                        nc.scalar.activation(
                            out=ex_t[:], in_=a_t[:],
                            func=mybir.ActivationFunctionType.Prelu,
                            alpha=NEG_SLOPE)
                        nc.scalar.activation(
                            out=a_t[:], in_=ex_t[:],
                            func=mybir.ActivationFunctionType.Exp)
                        # S = M * EX (in place on M)
                        nc.vector.tensor_tensor(out=m_t[:], in0=m_t[:],
                                                in1=a_t[:],
                                                op=mybir.AluOpType.mult)

                    for gg in range(G):
                        s = s0 + gg
                        if s >= NSUP:
                            continue
                        r0 = s * SUP
                        rows = min(SUP, NB - r0)
                        ps = epsum.tile([128, psw], FP, tag=f"eps{gg}")
                        if "no_matmul" in abl:
                            nc.vector.memset(ps[:], 1.0)
                        else:
                            for j in range(NJ):
                                jj = gg * NJ + j
                                nc.tensor.matmul(
                                    out=ps[:],
                                    lhsT=m_t[:, jj * SUP:(jj + 1) * SUP],
                                    rhs=x_t[:, jj, 0:psw],
                                    start=(j == 0),
                                    stop=(j == NJ - 1),
                                )

                        esum = eoutp.tile([128, 1], FP, tag="esum")
                        nc.vector.tensor_scalar_max(out=esum[:],
                                                    in0=ps[:, psw - 1:psw],
                                                    scalar1=1e-30)
                        rec = eoutp.tile([128, 1], FP, tag="rec")
                        nc.vector.reciprocal(out=rec[:], in_=esum[:])

                        o_t = eoutp.tile([128, f_out], FP if last else BF,
                                         tag="o")
                        if has_bias[li]:
                            nc.scalar.activation(
                                out=o_t[:rows], in_=ps[:rows, 0:f_out],
                                func=mybir.ActivationFunctionType.Copy,
                                scale=rec[:rows, 0:1])
                            nc.vector.tensor_tensor(
                                out=o_t[:rows], in0=o_t[:rows],
                                in1=bsb[:rows], op=mybir.AluOpType.add)
                            nc.vector.tensor_scalar_max(
                                out=o_t[:rows], in0=o_t[:rows], scalar1=0.0)
                        else:
                            nc.scalar.activation(
                                out=o_t[:rows], in_=ps[:rows, 0:f_out],
                                func=mybir.ActivationFunctionType.Relu,
                                scale=rec[:rows, 0:1])
                        dest = out_c if last else h_mid[li]
                        nc.sync.dma_start(out=dest[r0:r0 + rows, :],
                                          in_=o_t[:rows])

    nc.compile()
    return nc


_ABL = "full"  # ablation mode for ablate.py (timing experiments)

_CACHE = {}


def _get_program(cfg, has_bias):
    key = (cfg.key(), tuple(has_bias))
    if key not in _CACHE:
        _CACHE[key] = _build(cfg, has_bias)
    return _CACHE[key]


class _Runner:
    """PJRT execution with a cached jitted executable.

    run_bass_kernel_spmd re-traces/re-jits the shard_map body on every call
    (~1.2s) and re-uploads all inputs (~0.4s); caching the jitted callable and
    keeping inputs device-resident removes both, leaving dispatch + device
    execution.
    """

    def __init__(self, nc, n_cores):
        import jax
        from jax.sharding import Mesh, PartitionSpec, NamedSharding
        from jax.experimental.shard_map import shard_map
        from concourse.bass2jax import (
            _bass_exec_p, install_neuronx_cc_hook, partition_id_tensor)

        install_neuronx_cc_hook()
        self.jax = jax
        self.n_cores = n_cores
        partition_name = (nc.partition_id_tensor.name
                          if nc.partition_id_tensor else None)
        in_names, out_names, out_avals, zero_outs = [], [], [], []
        for alloc in nc.m.functions[0].allocations:
            if not isinstance(alloc, mybir.MemoryLocationSet):
                continue
            name = alloc.memorylocations[0].name
            if alloc.kind == "ExternalInput":
                if name != partition_name:
                    in_names.append(name)
            elif alloc.kind == "ExternalOutput":
                out_names.append(name)
                shape = tuple(alloc.tensor_shape)
                dtype = mybir.dt.np(alloc.dtype)
                out_avals.append(jax.core.ShapedArray(shape, dtype))
                zero_outs.append(np.zeros(shape, dtype))
        n_params = len(in_names)
        n_outs = len(out_avals)
        in_names_all = in_names + out_names
        if partition_name is not None:
            in_names_all.append(partition_name)
        donate = tuple(range(n_params, n_params + n_outs))

        def _body(*args):
            operands = list(args)
            if partition_name is not None:
                operands.append(partition_id_tensor())
            outs = _bass_exec_p.bind(
                *operands,
                out_avals=tuple(out_avals),
                in_names=tuple(in_names_all),
                out_names=tuple(out_names),
                lowering_input_output_aliases=(),
                sim_require_finite=True,
                sim_require_nnan=True,
                nc=nc,
            )
            return tuple(outs)

        devices = jax.devices()[:n_cores]
        mesh = Mesh(np.asarray(devices), ("core",))
        in_specs = (PartitionSpec("core"),) * (n_params + n_outs)
        out_specs = (PartitionSpec("core"),) * len(out_names)
        self.sharded = jax.jit(
            shard_map(_body, mesh=mesh, in_specs=in_specs,
                      out_specs=out_specs, check_rep=False),
            donate_argnums=donate, keep_unused=True)
        self.sharding = NamedSharding(mesh, PartitionSpec("core"))
        self.in_names = in_names
        self.out_names = out_names
        self.out_avals = out_avals
        self.zero_outs = zero_outs

    def put_inputs(self, in_maps):
        concat_in = [
            np.concatenate([np.asarray(m[name]) for m in in_maps], axis=0)
            for name in self.in_names
        ]
        return [self.jax.device_put(x, self.sharding) for x in concat_in]

    def make_zero_outs(self):
        return [
            self.jax.device_put(
                np.zeros((self.n_cores * z.shape[0], *z.shape[1:]), z.dtype),
                self.sharding)
            for z in self.zero_outs
        ]

    def call(self, dev_in, dev_zeros):
        return self.sharded(*dev_in, *dev_zeros)

    def to_results(self, outs):
        return [
            {name: np.asarray(outs[i]).reshape(
                self.n_cores, *self.out_avals[i].shape)[c]
             for i, name in enumerate(self.out_names)}
            for c in range(self.n_cores)
        ]

    def run(self, in_maps):
        dev_in = self.put_inputs(in_maps)
        outs = self.call(dev_in, self.make_zero_outs())
        self.jax.block_until_ready(outs)
        return self.to_results(outs)


_RUNNERS = {}


def _get_runner(nc, n_cores):
    if id(nc) not in _RUNNERS:
        _RUNNERS[id(nc)] = _Runner(nc, n_cores)
    return _RUNNERS[id(nc)]


class _RunnerResults:
    def __init__(self, results):
        self.results = results


def _run(cfg, prep, **run_kwargs):
    nc = _get_program(cfg, prep["has_bias"])
    try:
        runner = _get_runner(nc, cfg.C)
        return _RunnerResults(runner.run(prep["in_maps"]))
    except Exception:
        return run_bass_kernel_spmd(nc, prep["in_maps"], list(range(cfg.C)),
                                    **run_kwargs)


def _prepare(feat, src, dst, Ws, als, ars, bs, cfg, packed):
    jl, jh, meta, perms = packed
    assert jl == cfg.JL and jh == cfg.JH

    has_bias = tuple(bool(np.any(np.asarray(b) != 0)) for b in bs)
    iota = np.broadcast_to(
        np.arange(SUP, dtype=np.float32)[None, :], (128, SUP))

    feat = np.asarray(feat, np.float32)
    in_maps = []
    for c in range(cfg.C):
        block = feat[c * cfg.NB:(c + 1) * cfg.NB]
        m = {
            "feat_c": np.ascontiguousarray(block[perms[c]]).astype(BF_NP),
            "iota": iota.astype(BF_NP),
            "meta": meta[c],
        }
        for li in range(len(cfg.feats)):
            W = np.asarray(Ws[li], np.float32)
            al = np.asarray(als[li], np.float32)
            ar = np.asarray(ars[li], np.float32)
            m[f"waug{li}"] = np.ascontiguousarray(
                np.concatenate([W, (W @ al)[:, None], (W @ ar)[:, None]],
                               1)).astype(BF_NP)
            if has_bias[li]:
                m[f"bias{li}"] = np.broadcast_to(
                    np.asarray(bs[li], np.float32)[None, :],
                    (128, cfg.feats[li][1])).copy()
        in_maps.append(m)
    return {"in_maps": in_maps, "has_bias": has_bias, "perms": perms}


def _host_pack(feat, src, dst, Ws, als, ars, bs):
    """All host-side prep: permutation, edge packing, input staging."""
    feats = [(128, 128), (128, 128), (128, 16)]
    src = np.asarray(src)
    dst = np.asarray(dst)
    probe = _Cfg(N_NODES, N_CORES, feats, 1, 1)

    # First pass (dst only matters) to learn the device permutation, then
    # remap src ids into device rows and repack: the low/high int16 split
    # depends on the remapped src values.
    _, _, _, perms = _pack_edges(src, dst, probe)
    inv = np.zeros_like(perms)
    for c in range(N_CORES):
        inv[c][perms[c]] = np.arange(probe.NB)
    src_dev = (src // probe.NB) * probe.NB + \
        inv[src // probe.NB, src % probe.NB]
    jl, jh, meta, perms2 = _pack_edges(
        src_dev.astype(np.int32), dst, probe)
    assert np.array_equal(perms, perms2)

    cfg = _Cfg(N_NODES, N_CORES, feats, jl, jh)
    prep = _prepare(feat, src, dst, Ws, als, ars, bs, cfg,
                    (jl, jh, meta, perms))
    return cfg, prep


def kernel(feat, src, dst, W1, al1, ar1, b1, W2, al2, ar2, b2,
           W3, al3, ar3, b3):
    cfg, prep = _host_pack(feat, src, dst,
                           [W1, W2, W3], [al1, al2, al3], [ar1, ar2, ar3],
                           [b1, b2, b3])
    res = _run(cfg, prep).results
    perms = prep["perms"]
    out = np.zeros((N_NODES, cfg.feats[-1][1]), np.float32)
    for c in range(cfg.C):
        out[c * cfg.NB + perms[c]] = np.asarray(
            res[c]["out_c"], np.float32)
    return out


# revision 45
# speedup vs baseline: 1.1548x; 1.1548x over previous
"""3-layer GAT (DGL GATConv, 1 head) on Trainium2, sharded over 8 NeuronCores.

Strategy (graph/data parallel by destination node):
  * Nodes split into 8 blocks of 6250; each core owns edges whose dst falls in
    its block.  Within a core, dsts are bin-packed into 49 supertiles of <=128
    dsts each, balancing per-supertile edge counts (NJ = max blocks-of-128
    edges drops to ~the mean).  The node order within a core is therefore a
    permutation; all index remapping happens on host, the device only sees
    contiguous permuted rows, and the host unpermutes the final output.
  * Per layer each core computes a bf16 node-table shard with 512B rows
    [ft (f_out), 1.0, el, pad] (ft = h @ W, el = h @ (W @ al)); er = h@(W@ar)
    is kept in a local fp32 column.  Shards are AllGathered so every core
    holds the full 50000-row table in DRAM.
  * Edge stage per supertile: one big SWDGE dma_gather fetches all edge-source
    rows (512B each) in <=1024-index chunks — the int16 index limit splits the
    table into a low half (rows < 32768) and a high half (offset base).
    This replaces the per-128-row indirect DMA hot loop of the previous
    version (~1us fixed gpsimd cost per instruction).
  * ex = exp(leaky_relu(el_src + er_dst)); er_dst is broadcast to all
    partitions via a rank-1 matmul.  A one-hot mask (iota == shift) times ex
    forms the scatter matrix S; matmuls accumulate
    psum[128 dsts, f_out+1] += S_j^T @ X_j; the table's constant 1.0 column
    makes psum's last column the softmax denominator.  Softmax max-subtraction
    is skipped: logits are O(10), exp cannot overflow, result identical.
  * Epilogue: out = relu(psum * 1/esum) written to the core's permuted
    next-layer h block; layer 3 writes the external output shard (fp32).

Self-contained: hardcodes the problem shapes; host-side work is numpy only.
"""

import numpy as np
import ml_dtypes

import concourse.bass as bass
import concourse.mybir as mybir
import concourse.tile as tile
from concourse import bacc
from concourse.bass_utils import run_bass_kernel_spmd
from concourse.masks import make_identity

FP = mybir.dt.float32
BF = mybir.dt.bfloat16
I16 = mybir.dt.int16

BF_NP = ml_dtypes.bfloat16

N_NODES = 50000
N_CORES = 8
IN_F = 128
NEG_SLOPE = 0.2
SUP = 128           # dsts per supertile (= psum rows)
SPLIT = 32768       # int16 gather index limit: table low/high split row
CHUNK = 1024        # SWDGE descriptor-ring capacity (indices per dma_gather)
G = 2               # supertiles batched per edge-stage iteration


class _Cfg:
    def __init__(self, n_nodes, n_cores, feats, jl, jh):
        self.N = n_nodes
        self.C = n_cores
        self.NB = n_nodes // n_cores
        assert self.NB * n_cores == n_nodes
        self.feats = feats  # list of (F_in, F_out) per layer
        self.NSUP = -(-self.NB // SUP)
        self.JL = jl        # low-half blocks of 128 slots per supertile
        self.JH = jh        # high-half blocks
        self.NJ = jl + jh

    def table_width(self, li):
        # bf16 row [ft (f_out), one, el, pad...]: 512B rows for f_out=128,
        # 256B rows for f_out=16 (dma_gather needs elem_size % 256B == 0)
        f = self.feats[li][1]
        return 256 if f + 2 > 128 else 128

    @property
    def NITER(self):
        return -(-self.NSUP // G)

    def key(self):
        return (self.N, self.C, tuple(self.feats), self.JL, self.JH)


def _pack_edges(src, dst, cfg_probe):
    """Sort edges by dst block, bin-pack dsts into supertiles, split lo/hi.

    Returns (jl, jh, meta, perms): meta is int16
    [C, NITER, 128, G*NJ*8 + G*NJ] holding, per G-supertile iteration, the
    wrapped SWDGE gather indices for each supertile followed by the one-hot
    shift values as bf16 bits.  perms[c] maps device row -> original local
    dst id.
    """
    C, NB, NSUP = cfg_probe.C, cfg_probe.NB, cfg_probe.NSUP
    order = np.argsort(dst, kind="stable")
    src_s = src[order].astype(np.int64)
    dst_s = dst[order].astype(np.int64)
    core_lo = np.searchsorted(dst_s, np.arange(C) * NB)
    core_hi = np.searchsorted(dst_s, (np.arange(C) + 1) * NB)

    perms = np.zeros((C, NB), np.int64)
    percore = []
    jl = jh = 0
    for c in range(C):
        lo, hi = core_lo[c], core_hi[c]
        dloc = dst_s[lo:hi] - c * NB          # local dst of each edge
        sloc = src_s[lo:hi]
        deg = np.bincount(dloc, minlength=NB)
        # greedy balanced bin packing: heaviest dst first into lightest
        # feasible bin (<=128 dsts; bin NSUP-1 also capped to fit NB total)
        cap = np.full(NSUP, SUP, np.int64)
        cap[-1] = NB - (NSUP - 1) * SUP
        load = np.zeros(NSUP, np.int64)
        fill = np.zeros(NSUP, np.int64)
        bin_of = np.zeros(NB, np.int64)
        pos_of = np.zeros(NB, np.int64)
        for d in np.argsort(-deg, kind="stable"):
            b = np.argmin(np.where(fill < cap, load, np.iinfo(np.int64).max))
            bin_of[d] = b
            pos_of[d] = fill[b]
            fill[b] += 1
            load[b] += deg[d]
        # device row of original local dst d = bin_of*SUP + pos_of
        # (bins 0..NSUP-2 are full 128-dst bins; the last holds the rest)
        dev_row = bin_of * SUP + pos_of
        perm = np.zeros(NB, np.int64)
        perm[dev_row] = np.arange(NB)
        perms[c] = perm

        e_sup = bin_of[dloc]                  # supertile of each edge
        e_w = pos_of[dloc]                    # dst slot within supertile
        e_lo = sloc < SPLIT
        nlo = np.zeros(NSUP, np.int64)
        nhi = np.zeros(NSUP, np.int64)
        np.add.at(nlo, e_sup[e_lo], 1)
        np.add.at(nhi, e_sup[~e_lo], 1)
        jl = max(jl, -(-int(nlo.max()) // SUP))
        jh = max(jh, -(-int(nhi.max()) // SUP))
        percore.append((sloc, e_sup, e_w, e_lo))

    nj = jl + jh
    niter = -(-NSUP // G)
    shoff = G * nj * 8
    meta = np.zeros((C, niter, 128, G * nj * 9), np.int16)
    pad_bits = np.asarray(128.0, BF_NP).view(np.int16)
    meta[:, :, :, shoff:] = pad_bits
    plan = _chunk_plan(jl, jh)
    for c in range(C):
        sloc, e_sup, e_w, e_lo = percore[c]
        # slot index within supertile: low edges pack from 0, high edges
        # pack from JL*128
        key = e_sup * 4 + (~e_lo).astype(np.int64)  # group (sup, half)
        grp_order = np.argsort(key, kind="stable")
        key_sorted = key[grp_order]
        gstart = np.searchsorted(key_sorted, key)
        slot_in_grp = np.empty(len(key), np.int64)
        slot_in_grp[grp_order] = np.arange(len(key)) - gstart[grp_order]
        slot = np.where(e_lo, slot_in_grp, jl * 128 + slot_in_grp)
        flat_idx = np.zeros((NSUP, nj * 128), np.int64)
        val = np.where(e_lo, sloc, sloc - SPLIT)
        flat_idx[e_sup, slot] = val
        shift = np.full((NSUP, nj * 128), 128.0, np.float32)
        shift[e_sup, slot] = e_w[...]
        # per-iteration global block order [g0-low | g1-low | g0-hi | g1-hi]
        for it in range(niter):
            glob_idx = np.zeros(G * nj * 128, np.int64)
            glob_shift = np.full(G * nj * 128, 128.0, np.float32)
            for gg in range(G):
                s = it * G + gg
                if s >= NSUP:
                    continue
                glob_idx[gg * jl * 128:(gg + 1) * jl * 128] = \
                    flat_idx[s][0:jl * 128]
                glob_shift[gg * jl * 128:(gg + 1) * jl * 128] = \
                    shift[s][0:jl * 128]
                h0 = (G * jl + gg * jh) * 128
                glob_idx[h0:h0 + jh * 128] = flat_idx[s][jl * 128:]
                glob_shift[h0:h0 + jh * 128] = shift[s][jl * 128:]
            # wrapped SWDGE layout per gather chunk: within a chunk, index
            # i' sits at [16g + i'%16, chunk_col0 + i'//16] for g in 0..7
            row16 = np.zeros((16, G * nj * 8), np.int16)
            for b0, nb, _lo, _q in plan:
                vals = glob_idx[b0 * 128:(b0 + nb) * 128]
                row16[:, b0 * 8:(b0 + nb) * 8] = (
                    vals.reshape(nb * 8, 16).T.astype(np.int16))
            meta[c, it, :, 0:shoff] = np.tile(row16, (8, 1))
            # shift [k, b] = glob_shift[b*128+k], as bf16 bits
            sh = np.asarray(glob_shift.reshape(G * nj, 128).T,
                            BF_NP).view(np.int16)
            meta[c, it, :, shoff:] = sh
    return jl, jh, meta, perms


def _chunk_plan(jl, jh):
    """Gather chunking over an iteration's G*NJ blocks, low run first.

    The block order is [g0-low | g1-low | g0-high | g1-high]: each table
    half forms one contiguous run so gather chunks span supertiles, which
    minimizes instruction count at the 1024-descriptor ring limit.
    Returns (block0, nblocks, is_low, queue) with queues greedily balanced
    by descriptor count (descriptor generation parallelizes across the 4
    SWDGE queues).
    """
    runs = [(0, G * jl, True), (G * jl, G * jh, False)]
    chunks = []
    for start, n, lo in runs:
        done = 0
        while done < n:
            k = min(CHUNK // 128, n - done)
            chunks.append((start + done, k, lo))
            done += k
    loads = [0] * 4
    plan = []
    for b0, nb, lo in sorted(chunks, key=lambda c: -c[1]):
        q = min(range(4), key=lambda i: loads[i])
        loads[q] += nb
        plan.append((b0, nb, lo, q))
    plan.sort()
    return plan


def _build(cfg, has_bias):
    """Build + compile the (core-independent) Bass program."""
    nc = bacc.Bacc(
        "TRN2",
        target_bir_lowering=False,
        debug=False,
        num_devices=cfg.C,
        num_swdge_queues=4,
    )
    NB, NSUP, NJ, JL, JH = cfg.NB, cfg.NSUP, cfg.NJ, cfg.JL, cfg.JH
    NL = len(cfg.feats)

    NITER = cfg.NITER
    SHOFF = G * NJ * 8
    feat_c = nc.dram_tensor("feat_c", [NB, IN_F], BF, kind="ExternalInput")
    iota_in = nc.dram_tensor("iota", [128, SUP], BF, kind="ExternalInput")
    meta_in = nc.dram_tensor("meta", [NITER, 128, G * NJ * 9], I16,
                             kind="ExternalInput")
    waug = [
        nc.dram_tensor(f"waug{li}", [cfg.feats[li][0], cfg.feats[li][1] + 2],
                       BF, kind="ExternalInput")
        for li in range(NL)
    ]
    bias_in = [
        nc.dram_tensor(f"bias{li}", [128, cfg.feats[li][1]], FP,
                       kind="ExternalInput")
        if has_bias[li] else None
        for li in range(NL)
    ]

    tbl_shard = [
        nc.dram_tensor(f"tbl_shard{li}", [NB, cfg.table_width(li)], BF)
        for li in range(NL)
    ]
    shared_kw = {"addr_space": "Shared"} if cfg.C > 4 else {}
    tbl_full = [
        nc.dram_tensor(f"tbl_full{li}", [cfg.N, cfg.table_width(li)], BF,
                       **shared_kw)
        for li in range(NL)
    ]
    er_own = [nc.dram_tensor(f"er_own{li}", [NB, 1], BF) for li in range(NL)]
    h_mid = [
        nc.dram_tensor(f"h_mid{li}", [NB, cfg.feats[li][1]], BF)
        for li in range(NL - 1)
    ]
    out_c = nc.dram_tensor("out_c", [NB, cfg.feats[-1][1]], FP,
                           kind="ExternalOutput")

    n_row_tiles = -(-NB // 128)
    replica = [list(range(cfg.C))]
    plan = _chunk_plan(JL, JH)

    with tile.TileContext(nc, num_cores=cfg.C) as tc:
        with (
            tc.tile_pool(name="const", bufs=1) as constp,
            tc.tile_pool(name="nodein", bufs=3) as nodein,
            tc.tile_pool(name="nodet", bufs=2) as nodet,
            tc.tile_pool(name="nodepsum", bufs=1, space="PSUM") as nodepsum,
            tc.tile_pool(name="stage", bufs=4) as stagep,
            tc.tile_pool(name="erp", bufs=1) as erp,
            tc.tile_pool(name="idx", bufs=4) as idxp,
            tc.tile_pool(name="xg", bufs=2) as xgp,
            tc.tile_pool(name="aex", bufs=1) as aexp,
            tc.tile_pool(name="sm", bufs=2) as smp,
            tc.tile_pool(name="ebp", bufs=2, space="PSUM") as ebp,
            tc.tile_pool(name="epsum", bufs=2, space="PSUM") as epsum,
            tc.tile_pool(name="eout", bufs=4) as eoutp,
        ):
            ident = constp.tile([128, 128], BF, tag="ident")
            make_identity(nc, ident[:])
            iota_sb = constp.tile([128, SUP], BF, tag="iota")
            nc.sync.dma_start(out=iota_sb[:], in_=iota_in[:])
            ones_row = constp.tile([1, SUP], BF, tag="ones")
            nc.vector.memset(ones_row[:], 1.0)
            ones_col = constp.tile([128, 1], BF, tag="ones_col")
            nc.vector.memset(ones_col[:], 1.0)

            for li in range(NL):
                f_in, f_out = cfg.feats[li]
                tw = cfg.table_width(li)

                wsb = constp.tile([f_in, f_out + 2], BF, tag=f"waug{li}")
                nc.sync.dma_start(out=wsb[:], in_=waug[li][:])
                if has_bias[li]:
                    bsb = constp.tile([128, f_out], FP, tag=f"bias{li}")
                    nc.sync.dma_start(out=bsb[:], in_=bias_in[li][:])

                # ---- node stage: own block rows -> table shard + er column
                # ps2 is a bf16 psum tile [ft, el, er]; the table row
                # [ft, one, el] is written straight from psum by DMA (no
                # Act copies): ft -> cols 0:f_out, el -> col f_out+1,
                # er -> er_own, and the constant-one column from a tile.
                hsrc = feat_c if li == 0 else h_mid[li - 1]
                for t in range(n_row_tiles):
                    r0 = t * 128
                    rows = min(128, NB - r0)
                    h_t = nodein.tile([128, f_in], BF, tag="h")
                    nc.sync.dma_start(out=h_t[:rows], in_=hsrc[r0:r0 + rows, :])
                    ps_t = nodepsum.tile([f_in, 128], BF, tag="pT")
                    nc.tensor.transpose(out=ps_t[:, :rows], in_=h_t[:rows],
                                        identity=ident[:rows, :rows])
                    hT = nodet.tile([f_in, 128], BF, tag="hT")
                    nc.scalar.copy(out=hT[:, :rows], in_=ps_t[:, :rows])
                    ps2 = nodepsum.tile([128, f_out + 2], FP, tag="p2")
                    nc.tensor.matmul(out=ps2[:rows], lhsT=hT[:, :rows],
                                     rhs=wsb[:], start=True, stop=True)
                    st = stagep.tile([128, f_out + 2], BF, tag="st")
                    nc.scalar.copy(out=st[:rows, 0:f_out],
                                   in_=ps2[:rows, 0:f_out])
                    nc.vector.memset(st[:rows, f_out:f_out + 1], 1.0)
                    nc.scalar.copy(out=st[:rows, f_out + 1:f_out + 2],
                                   in_=ps2[:rows, f_out:f_out + 1])
                    nc.sync.dma_start(
                        out=tbl_shard[li][r0:r0 + rows, 0:f_out + 2],
                        in_=st[:rows])
                    er_st = stagep.tile([128, 1], BF, tag="er_st")
                    nc.scalar.copy(out=er_st[:rows],
                                   in_=ps2[:rows, f_out + 1:f_out + 2])
                    nc.sync.dma_start(out=er_own[li][r0:r0 + rows, :],
                                      in_=er_st[:rows])

                # ---- all-gather the node table ----
                if "no_collective" not in _ABL:
                    nc.gpsimd.collective_compute(
                        "AllGather",
                        mybir.AluOpType.bypass,
                        replica_groups=replica,
                        ins=[tbl_shard[li][:]],
                        outs=[tbl_full[li][:]],
                    )

                abl = set(_ABL.split("+"))
                # per-layer er preload: one [1, NITER*G*SUP] row, padded 0
                er_all = erp.tile([1, NITER * G * SUP], BF, tag="er_all")
                nc.vector.memset(er_all[:], 0.0)
                nc.sync.dma_start(out=er_all[:1, 0:NB],
                                  in_=er_own[li][:, 0][None, :])

                last = li == NL - 1
                el_c = f_out + 1
                psw = f_out + 1
                # ---- edge stage: G supertiles per iteration ----
                for it in range(NITER):
                    s0 = it * G
                    meta_t = idxp.tile([128, G * NJ * 9], I16, tag="meta")
                    nc.sync.dma_start(out=meta_t[:], in_=meta_in[it])
                    x_t = xgp.tile([128, G * NJ, tw], BF, tag="x")
                    if "no_gather" in abl:
                        nc.vector.memset(
                            x_t[:].rearrange("p j c -> p (j c)"), 0.0)
                    else:
                        for b0, nb, is_lo, qn in plan:
                            tbl_ap = (tbl_full[li][0:SPLIT, :] if is_lo
                                      else tbl_full[li][SPLIT:cfg.N, :])
                            nc.gpsimd.dma_gather(
                                out_ap=x_t[:, b0:b0 + nb, :],
                                in_ap=tbl_ap,
                                idxs_ap=meta_t[:, b0 * 8:(b0 + nb) * 8],
                                num_idxs=nb * 128,
                                num_idxs_reg=nb * 128,
                                elem_size=tw,
                                queue_num=qn,
                            )

                    # er for these supertiles' dsts -> all partitions
                    eb_ps = ebp.tile([128, G * SUP], FP, tag="eb")
                    nc.tensor.matmul(
                        out=eb_ps[:], lhsT=ones_row[:],
                        rhs=er_all[0:1, s0 * SUP:(s0 + G) * SUP],
                        start=True, stop=True)

                    # A = el_src + er_dst, per (supertile, table-half) run
                    # (er varies by g; x blocks of g form two runs)
                    a_t = aexp.tile([128, G * NJ * SUP], BF, tag="a")
                    if "no_ex" not in abl:
                        for gg in range(G):
                            for b0, nb in ((gg * JL, JL),
                                           (G * JL + gg * JH, JH)):
                                a3 = a_t[:, b0 * SUP:(b0 + nb) * SUP] \
                                    .rearrange("p (j w) -> p j w", w=SUP)
                                nc.vector.tensor_tensor(
                                    out=a3,
                                    in0=x_t[:, b0:b0 + nb,
                                            el_c:el_c + 1].to_broadcast(
                                        [128, nb, SUP]),
                                    in1=eb_ps[:, None,
                                              gg * SUP:(gg + 1) * SUP]
                                    .to_broadcast([128, nb, SUP]),
                                    op=mybir.AluOpType.add,
                                )
                    # M = (iota == shift)
                    m_t = smp.tile([128, G * NJ * SUP], BF, tag="m")
                    m3 = m_t[:].rearrange("p (j w) -> p j w", w=SUP)
                    nc.vector.tensor_tensor(
                        out=m3,
                        in0=iota_sb[:, None, :].to_broadcast(
                            [128, G * NJ, SUP]),
                        in1=meta_t[:, SHOFF:].bitcast(BF)[:, :, None]
                        .to_broadcast([128, G * NJ, SUP]),
                        op=mybir.AluOpType.is_equal,
                    )
                    if "no_ex" not in abl:
                        # EX = exp(leaky_relu(A)) on the scalar engine
                        ex_t = aexp.tile([128, G * NJ * SUP], BF, tag="exv")
                        nc.scalar.activation(
                            out=ex_t[:], in_=a_t[:],
                            func=mybir.ActivationFunctionType.Prelu,
                            alpha=NEG_SLOPE)
                        nc.scalar.activation(
                            out=a_t[:], in_=ex_t[:],
                            func=mybir.ActivationFunctionType.Exp)
                        # S = M * EX (in place on M)
                        nc.vector.tensor_tensor(out=m_t[:], in0=m_t[:],
                                                in1=a_t[:],
                                                op=mybir.AluOpType.mult)

                    for gg in range(G):
                        s = s0 + gg
                        if s >= NSUP:
                            continue
                        r0 = s * SUP
                        rows = min(SUP, NB - r0)
                        ps = epsum.tile([128, psw], FP, tag=f"eps{gg}")
                        if "no_matmul" in abl:
                            nc.vector.memset(ps[:], 1.0)
                        else:
                            blocks_g = (
                                list(range(gg * JL, (gg + 1) * JL)) +
                                list(range(G * JL + gg * JH,
                                           G * JL + (gg + 1) * JH)))
                            for j, jj in enumerate(blocks_g):
                                nc.tensor.matmul(
                                    out=ps[:],
                                    lhsT=m_t[:, jj * SUP:(jj + 1) * SUP],
                                    rhs=x_t[:, jj, 0:psw],
                                    start=(j == 0),
                                    stop=(j == NJ - 1),
                                )

                        esum = eoutp.tile([128, 1], FP, tag="esum")
                        nc.vector.tensor_scalar_max(out=esum[:],
                                                    in0=ps[:, psw - 1:psw],
                                                    scalar1=1e-30)
                        rec = eoutp.tile([128, 1], FP, tag="rec")
                        nc.vector.reciprocal(out=rec[:], in_=esum[:])

                        o_t = eoutp.tile([128, f_out], FP if last else BF,
                                         tag="o")
                        if has_bias[li]:
                            nc.scalar.activation(
                                out=o_t[:rows], in_=ps[:rows, 0:f_out],
                                func=mybir.ActivationFunctionType.Copy,
                                scale=rec[:rows, 0:1])
                            nc.vector.tensor_tensor(
                                out=o_t[:rows], in0=o_t[:rows],
                                in1=bsb[:rows], op=mybir.AluOpType.add)
                            nc.vector.tensor_scalar_max(
                                out=o_t[:rows], in0=o_t[:rows], scalar1=0.0)
                        else:
                            nc.scalar.activation(
                                out=o_t[:rows], in_=ps[:rows, 0:f_out],
                                func=mybir.ActivationFunctionType.Relu,
                                scale=rec[:rows, 0:1])
                        dest = out_c if last else h_mid[li]
                        nc.sync.dma_start(out=dest[r0:r0 + rows, :],
                                          in_=o_t[:rows])

    nc.compile()
    return nc


_ABL = "full"  # ablation mode for ablate.py (timing experiments)

_CACHE = {}


def _get_program(cfg, has_bias):
    key = (cfg.key(), tuple(has_bias))
    if key not in _CACHE:
        _CACHE[key] = _build(cfg, has_bias)
    return _CACHE[key]


class _Runner:
    """PJRT execution with a cached jitted executable.

    run_bass_kernel_spmd re-traces/re-jits the shard_map body on every call
    (~1.2s) and re-uploads all inputs (~0.4s); caching the jitted callable and
    keeping inputs device-resident removes both, leaving dispatch + device
    execution.
    """

    def __init__(self, nc, n_cores):
        import jax
        from jax.sharding import Mesh, PartitionSpec, NamedSharding
        from jax.experimental.shard_map import shard_map
        from concourse.bass2jax import (
            _bass_exec_p, install_neuronx_cc_hook, partition_id_tensor)

        install_neuronx_cc_hook()
        self.jax = jax
        self.n_cores = n_cores
        partition_name = (nc.partition_id_tensor.name
                          if nc.partition_id_tensor else None)
        in_names, out_names, out_avals, zero_outs = [], [], [], []
        for alloc in nc.m.functions[0].allocations:
            if not isinstance(alloc, mybir.MemoryLocationSet):
                continue
            name = alloc.memorylocations[0].name
            if alloc.kind == "ExternalInput":
                if name != partition_name:
                    in_names.append(name)
            elif alloc.kind == "ExternalOutput":
                out_names.append(name)
                shape = tuple(alloc.tensor_shape)
                dtype = mybir.dt.np(alloc.dtype)
                out_avals.append(jax.core.ShapedArray(shape, dtype))
                zero_outs.append(np.zeros(shape, dtype))
        n_params = len(in_names)
        n_outs = len(out_avals)
        in_names_all = in_names + out_names
        if partition_name is not None:
            in_names_all.append(partition_name)
        donate = tuple(range(n_params, n_params + n_outs))

        def _body(*args):
            operands = list(args)
            if partition_name is not None:
                operands.append(partition_id_tensor())
            outs = _bass_exec_p.bind(
                *operands,
                out_avals=tuple(out_avals),
                in_names=tuple(in_names_all),
                out_names=tuple(out_names),
                lowering_input_output_aliases=(),
                sim_require_finite=True,
                sim_require_nnan=True,
                nc=nc,
            )
            return tuple(outs)

        devices = jax.devices()[:n_cores]
        mesh = Mesh(np.asarray(devices), ("core",))
        in_specs = (PartitionSpec("core"),) * (n_params + n_outs)
        out_specs = (PartitionSpec("core"),) * len(out_names)
        self.sharded = jax.jit(
            shard_map(_body, mesh=mesh, in_specs=in_specs,
                      out_specs=out_specs, check_rep=False),
            donate_argnums=donate, keep_unused=True)
        self.sharding = NamedSharding(mesh, PartitionSpec("core"))
        self.in_names = in_names
        self.out_names = out_names
        self.out_avals = out_avals
        self.zero_outs = zero_outs

    def put_inputs(self, in_maps):
        concat_in = [
            np.concatenate([np.asarray(m[name]) for m in in_maps], axis=0)
            for name in self.in_names
        ]
        return [self.jax.device_put(x, self.sharding) for x in concat_in]

    def make_zero_outs(self):
        return [
            self.jax.device_put(
                np.zeros((self.n_cores * z.shape[0], *z.shape[1:]), z.dtype),
                self.sharding)
            for z in self.zero_outs
        ]

    def call(self, dev_in, dev_zeros):
        return self.sharded(*dev_in, *dev_zeros)

    def to_results(self, outs):
        return [
            {name: np.asarray(outs[i]).reshape(
                self.n_cores, *self.out_avals[i].shape)[c]
             for i, name in enumerate(self.out_names)}
            for c in range(self.n_cores)
        ]

    def run(self, in_maps):
        dev_in = self.put_inputs(in_maps)
        outs = self.call(dev_in, self.make_zero_outs())
        self.jax.block_until_ready(outs)
        return self.to_results(outs)


_RUNNERS = {}


def _get_runner(nc, n_cores):
    if id(nc) not in _RUNNERS:
        _RUNNERS[id(nc)] = _Runner(nc, n_cores)
    return _RUNNERS[id(nc)]


class _RunnerResults:
    def __init__(self, results):
        self.results = results


def _run(cfg, prep, **run_kwargs):
    nc = _get_program(cfg, prep["has_bias"])
    try:
        runner = _get_runner(nc, cfg.C)
        return _RunnerResults(runner.run(prep["in_maps"]))
    except Exception:
        return run_bass_kernel_spmd(nc, prep["in_maps"], list(range(cfg.C)),
                                    **run_kwargs)


def _prepare(feat, src, dst, Ws, als, ars, bs, cfg, packed):
    jl, jh, meta, perms = packed
    assert jl == cfg.JL and jh == cfg.JH

    has_bias = tuple(bool(np.any(np.asarray(b) != 0)) for b in bs)
    iota = np.broadcast_to(
        np.arange(SUP, dtype=np.float32)[None, :], (128, SUP))

    feat = np.asarray(feat, np.float32)
    in_maps = []
    for c in range(cfg.C):
        block = feat[c * cfg.NB:(c + 1) * cfg.NB]
        m = {
            "feat_c": np.ascontiguousarray(block[perms[c]]).astype(BF_NP),
            "iota": iota.astype(BF_NP),
            "meta": meta[c],
        }
        for li in range(len(cfg.feats)):
            W = np.asarray(Ws[li], np.float32)
            al = np.asarray(als[li], np.float32)
            ar = np.asarray(ars[li], np.float32)
            m[f"waug{li}"] = np.ascontiguousarray(
                np.concatenate([W, (W @ al)[:, None], (W @ ar)[:, None]],
                               1)).astype(BF_NP)
            if has_bias[li]:
                m[f"bias{li}"] = np.broadcast_to(
                    np.asarray(bs[li], np.float32)[None, :],
                    (128, cfg.feats[li][1])).copy()
        in_maps.append(m)
    return {"in_maps": in_maps, "has_bias": has_bias, "perms": perms}


def _host_pack(feat, src, dst, Ws, als, ars, bs):
    """All host-side prep: permutation, edge packing, input staging."""
    feats = [(128, 128), (128, 128), (128, 16)]
    src = np.asarray(src)
    dst = np.asarray(dst)
    probe = _Cfg(N_NODES, N_CORES, feats, 1, 1)

    # First pass (dst only matters) to learn the device permutation, then
    # remap src ids into device rows and repack: the low/high int16 split
    # depends on the remapped src values.
    _, _, _, perms = _pack_edges(src, dst, probe)
    inv = np.zeros_like(perms)
    for c in range(N_CORES):
        inv[c][perms[c]] = np.arange(probe.NB)
    src_dev = (src // probe.NB) * probe.NB + \
        inv[src // probe.NB, src % probe.NB]
    jl, jh, meta, perms2 = _pack_edges(
        src_dev.astype(np.int32), dst, probe)
    assert np.array_equal(perms, perms2)

    cfg = _Cfg(N_NODES, N_CORES, feats, jl, jh)
    prep = _prepare(feat, src, dst, Ws, als, ars, bs, cfg,
                    (jl, jh, meta, perms))
    return cfg, prep


def kernel(feat, src, dst, W1, al1, ar1, b1, W2, al2, ar2, b2,
           W3, al3, ar3, b3):
    cfg, prep = _host_pack(feat, src, dst,
                           [W1, W2, W3], [al1, al2, al3], [ar1, ar2, ar3],
                           [b1, b2, b3])
    res = _run(cfg, prep).results
    perms = prep["perms"]
    out = np.zeros((N_NODES, cfg.feats[-1][1]), np.float32)
    for c in range(cfg.C):
        out[c * cfg.NB + perms[c]] = np.asarray(
            res[c]["out_c"], np.float32)
    return out
